# revision 6
# baseline (speedup 1.0000x reference)
"""MetaGL retrieval-knn kernel for 8x TRN2 NeuronCores.

Strategy (per sharding hint): shard query rows of every knn_edges(X, Y) call
across the 8 cores; replicate Y, W, Pm. Each core:
  phase 0: row-normalize + PE-transpose Y matrices to DRAM ([dim,rows] layout),
           transpose its own X row-slices and W.
  phase 1: for each of the 5 knn calls, row-block sims = Xt.T @ Yt_n via fp32
           matmuls, then exact top-30 per row with DVE max8/max_index/
           match_replace (jax.lax.top_k semantics: descending, ties -> lower
           index first).
  phase 2: graph_emb = cat(M,Pg) @ W.T + b computed transposed (Gt), then
           scores = G @ Pm.T, all on-chip.
Host only slices inputs, concatenates per-core outputs, and emits the
data-independent u = repeat(arange(n), 30) index arrays.
"""
import os
import sys
import time

sys.path.insert(0, "/opt/trn_rl_repo")

import numpy as np

import concourse.bass as bass
import concourse.mybir as mybir
import concourse.tile as tile
from concourse import bacc, bass_utils
from concourse.masks import make_identity

F32 = mybir.dt.float32
U32 = mybir.dt.uint32
AF = mybir.ActivationFunctionType

NC = 8
KNN_K = 30
EPS = 1e-8
NEG = -1e30

N_GRAPHS = 8192
N_MODELS = 2048
IN_DIM = 512          # input_dim
M_DIM = 1024          # 2 * metafeats_dim
W_OUT = IN_DIM        # rows of W
W_IN = M_DIM + IN_DIM  # cols of W (1536)


def build_program(n_graphs=N_GRAPHS, n_models=N_MODELS):
    gx = n_graphs // NC   # graph query rows per core
    mx = n_models // NC   # model query rows per core
    assert gx % 128 == 0 and mx % 128 == 0

    nc = bacc.Bacc("TRN2", target_bir_lowering=False, debug=False,
                   num_devices=NC)

    def din(name, shape):
        return nc.dram_tensor(name, shape, F32, kind="ExternalInput").ap()

    def dout(name, shape, dt=F32):
        return nc.dram_tensor(name, shape, dt, kind="ExternalOutput").ap()

    def dint(name, shape):
        return nc.dram_tensor(name, shape, F32).ap()

    M = din("M", [n_graphs, M_DIM])
    Pg = din("Pg", [n_graphs, IN_DIM])
    Pm = din("Pm", [n_models, IN_DIM])
    W = din("W", [W_OUT, W_IN])
    B = din("b", [W_OUT])
    Xm = din("Xm", [gx, M_DIM])
    Xpg = din("Xpg", [gx, IN_DIM])
    Xpm = din("Xpm", [mx, IN_DIM])

    scores_o = dout("scores", [gx, n_models])
    v_o = {
        "mg2g": dout("v_mg2g", [gx, KNN_K], U32),
        "pg2g": dout("v_pg2g", [gx, KNN_K], U32),
        "pg2m": dout("v_pg2m", [gx, KNN_K], U32),
        "pm2g": dout("v_pm2g", [mx, KNN_K], U32),
        "pm2m": dout("v_pm2m", [mx, KNN_K], U32),
    }

    # transposed [k_chunk, 128, rows] intermediates
    mt_n = dint("mt_n", [M_DIM // 128, 128, n_graphs])
    pgt_n = dint("pgt_n", [IN_DIM // 128, 128, n_graphs])
    pmt_n = dint("pmt_n", [IN_DIM // 128, 128, n_models])
    pmt_r = dint("pmt_r", [IN_DIM // 128, 128, n_models])
    xmt = dint("xmt", [M_DIM // 128, 128, gx])
    xpgt = dint("xpgt", [IN_DIM // 128, 128, gx])
    xpmt = dint("xpmt", [IN_DIM // 128, 128, mx])
    wt = dint("wt", [W_IN // 128, 128, W_OUT])

    with tile.TileContext(nc) as tc:
        with tc.tile_pool(name="ident", bufs=1) as identp:
            ident = identp.tile([128, 128], F32)
            make_identity(nc, ident[:])

            # ---------------- phase 0: normalize + transpose ----------------
            with tc.tile_pool(name="p0in", bufs=8) as p0in, \
                 tc.tile_pool(name="p0sc", bufs=2) as p0sc, \
                 tc.tile_pool(name="p0st", bufs=8) as p0st, \
                 tc.tile_pool(name="p0n", bufs=8) as p0n, \
                 tc.tile_pool(name="p0ps", bufs=8, space="PSUM") as p0ps, \
                 tc.tile_pool(name="p0out", bufs=4) as p0out:

                def norm_transpose(src, rows, dim, dst, normalize):
                    nk = dim // 128
                    for r0 in range(0, rows, 512):
                        nt = min(4, (rows - r0) // 128)
                        rtiles = []
                        for t in range(nt):
                            ti = p0in.tile([128, dim], F32, tag="p0in")
                            nc.sync.dma_start(
                                ti[:], src[r0 + t * 128:r0 + (t + 1) * 128, :])
                            if normalize:
                                scr = p0sc.tile([128, dim], F32, tag="p0sc")
                                ssq = p0st.tile([128, 1], F32, tag="ssq")
                                nc.scalar.activation(scr[:], ti[:], AF.Square,
                                                     accum_out=ssq[:])
                                nrm = p0st.tile([128, 1], F32, tag="nrm")
                                nc.scalar.sqrt(nrm[:], ssq[:])
                                nc.vector.tensor_scalar_max(nrm[:], nrm[:], EPS)
                                rn = p0st.tile([128, 1], F32, tag="rn")
                                nc.vector.reciprocal(rn[:], nrm[:])
                                tn = p0n.tile([128, dim], F32, tag="p0n")
                                nc.scalar.mul(tn[:], ti[:], rn[:])
                                rtiles.append(tn)
                            else:
                                rtiles.append(ti)
                        for k in range(nk):
                            ps = p0ps.tile([128, nt * 128], F32, tag="p0ps")
                            for t in range(nt):
                                nc.tensor.transpose(
                                    ps[:, t * 128:(t + 1) * 128],
                                    rtiles[t][:, k * 128:(k + 1) * 128],
                                    ident[:])
                            ob = p0out.tile([128, nt * 128], F32, tag="p0out")
                            nc.scalar.copy(ob[:], ps[:])
                            nc.sync.dma_start(dst[k][:, r0:r0 + nt * 128], ob[:])

                norm_transpose(M, n_graphs, M_DIM, mt_n, True)
                norm_transpose(Pg, n_graphs, IN_DIM, pgt_n, True)
                norm_transpose(Pm, n_models, IN_DIM, pmt_n, True)
                norm_transpose(Pm, n_models, IN_DIM, pmt_r, False)
                norm_transpose(Xm, gx, M_DIM, xmt, False)
                norm_transpose(Xpg, gx, IN_DIM, xpgt, False)
                norm_transpose(Xpm, mx, IN_DIM, xpmt, False)
                norm_transpose(W, W_OUT, W_IN, wt, False)

            # ---------------- phase 1: sims + topk per call ----------------
            def knn_call(xt, yt, x_rows, y_rows, dim, out_v, gs):
                nk = dim // 128
                nblocks = x_rows // 128
                nj = y_rows // 512
                with tc.tile_pool(name="lt", bufs=2 * gs * nk) as ltp, \
                     tc.tile_pool(name="rt", bufs=2 * nk) as rtp, \
                     tc.tile_pool(name="sims", bufs=gs) as simsp, \
                     tc.tile_pool(name="mmps", bufs=8, space="PSUM") as psp, \
                     tc.tile_pool(name="tk", bufs=8) as tkp:
                    for g0 in range(0, nblocks, gs):
                        blocks = range(g0, min(g0 + gs, nblocks))
                        lts = {}
                        for b in blocks:
                            for k in range(nk):
                                tl = ltp.tile([128, 128], F32, tag="lt")
                                nc.sync.dma_start(
                                    tl[:], xt[k][:, b * 128:(b + 1) * 128])
                                lts[(b, k)] = tl
                        sts = {b: simsp.tile([128, y_rows], F32, tag="sims",
                                             name=f"sims{b}")
                               for b in blocks}
                        for j in range(nj):
                            rts = []
                            for k in range(nk):
                                tr = rtp.tile([128, 512], F32, tag="rt")
                                nc.sync.dma_start(
                                    tr[:], yt[k][:, j * 512:(j + 1) * 512])
                                rts.append(tr)
                            for b in blocks:
                                ps = psp.tile([128, 512], F32, tag="mmps")
                                for k in range(nk):
                                    nc.tensor.matmul(ps[:], lts[(b, k)][:],
                                                     rts[k][:],
                                                     start=(k == 0),
                                                     stop=(k == nk - 1))
                                nc.scalar.copy(
                                    sts[b][:, j * 512:(j + 1) * 512], ps[:])
                        for b in blocks:
                            st = sts[b]
                            vals = tkp.tile([128, 32], F32, tag="tkv")
                            idx = tkp.tile([128, 32], U32, tag="tki")
                            for r in range(4):
                                v8 = vals[:, r * 8:(r + 1) * 8]
                                nc.vector.max(v8, st[:])
                                nc.vector.max_index(
                                    idx[:, r * 8:(r + 1) * 8], v8, st[:])
                                if r < 3:
                                    nc.vector.match_replace(st[:], v8, st[:],
                                                            NEG)
                            nc.sync.dma_start(
                                out_v[b * 128:(b + 1) * 128, :],
                                idx[:, :KNN_K])

            knn_call(xmt, mt_n, gx, n_graphs, M_DIM, v_o["mg2g"], 4)
            knn_call(xpgt, pgt_n, gx, n_graphs, IN_DIM, v_o["pg2g"], 4)
            knn_call(xpgt, pmt_n, gx, n_models, IN_DIM, v_o["pg2m"],
                     min(8, gx // 128))
            knn_call(xpmt, pgt_n, mx, n_graphs, IN_DIM, v_o["pm2g"], 2)
            knn_call(xpmt, pmt_n, mx, n_models, IN_DIM, v_o["pm2m"], 2)

            # ---------------- phase 2: graph_emb + scores ----------------
            nkw = W_IN // 128      # 12
            nkd = W_OUT // 128     # 4
            with tc.tile_pool(name="g2w", bufs=nkw) as g2w, \
                 tc.tile_pool(name="g2a", bufs=nkw) as g2a, \
                 tc.tile_pool(name="g2b", bufs=nkd) as g2b, \
                 tc.tile_pool(name="g2g", bufs=nkd) as g2g, \
                 tc.tile_pool(name="g2pm", bufs=nkd) as g2pm, \
                 tc.tile_pool(name="g2ps", bufs=8, space="PSUM") as g2ps, \
                 tc.tile_pool(name="g2s", bufs=2) as g2s:
                wts = []
                for k in range(nkw):
                    t = g2w.tile([128, W_OUT], F32, tag="g2w")
                    nc.sync.dma_start(t[:], wt[k][:, :])
                    wts.append(t)
                ats = []
                for k in range(nkw):
                    t = g2a.tile([128, gx], F32, tag="g2a")
                    src = xmt[k] if k < M_DIM // 128 else xpgt[k - M_DIM // 128]
                    nc.sync.dma_start(t[:], src[:, :])
                    ats.append(t)
                bts = []
                for d in range(nkd):
                    t = g2b.tile([128, 1], F32, tag="g2b")
                    nc.sync.dma_start(t[:], B[d * 128:(d + 1) * 128].unsqueeze(1))
                    bts.append(t)
                gts = []
                cs = min(512, gx)
                for d in range(nkd):
                    gt = g2g.tile([128, gx], F32, tag="g2g")
                    for c in range(gx // cs):
                        ps = g2ps.tile([128, cs], F32, tag="g2ps")
                        for k in range(nkw):
                            nc.tensor.matmul(
                                ps[:], wts[k][:, d * 128:(d + 1) * 128],
                                ats[k][:, c * cs:(c + 1) * cs],
                                start=(k == 0), stop=(k == nkw - 1))
                        nc.vector.tensor_scalar(
                            gt[:, c * cs:(c + 1) * cs], ps[:], bts[d][:],
                            None, op0=mybir.AluOpType.add)
                    gts.append(gt)
                pmts = []
                for k in range(nkd):
                    t = g2pm.tile([128, n_models], F32, tag="g2pm")
                    nc.sync.dma_start(t[:], pmt_r[k][:, :])
                    pmts.append(t)
                for i in range(gx // 128):
                    ssb = g2s.tile([128, n_models], F32, tag="g2s")
                    for n in range(n_models // 512):
                        ps = g2ps.tile([128, 512], F32, tag="g2ps")
                        for k in range(nkd):
                            nc.tensor.matmul(
                                ps[:], gts[k][:, i * 128:(i + 1) * 128],
                                pmts[k][:, n * 512:(n + 1) * 512],
                                start=(k == 0), stop=(k == nkd - 1))
                        nc.scalar.copy(ssb[:, n * 512:(n + 1) * 512], ps[:])
                    nc.sync.dma_start(scores_o[i * 128:(i + 1) * 128, :],
                                      ssb[:])

    nc.compile()
    return nc


_prog_cache = {}


def _get_program(n_graphs, n_models):
    key = (n_graphs, n_models)
    if key not in _prog_cache:
        _prog_cache[key] = build_program(n_graphs, n_models)
    return _prog_cache[key]


def kernel(M, Pg, Pm, W, b, _trace=False):
    M = np.ascontiguousarray(M, dtype=np.float32)
    Pg = np.ascontiguousarray(Pg, dtype=np.float32)
    Pm = np.ascontiguousarray(Pm, dtype=np.float32)
    W = np.ascontiguousarray(W, dtype=np.float32)
    b = np.ascontiguousarray(b, dtype=np.float32)
    n_graphs = M.shape[0]
    n_models = Pm.shape[0]
    gx = n_graphs // NC
    mx = n_models // NC

    nc = _get_program(n_graphs, n_models)
    in_maps = []
    for c in range(NC):
        in_maps.append({
            "M": M, "Pg": Pg, "Pm": Pm, "W": W, "b": b,
            "Xm": M[c * gx:(c + 1) * gx],
            "Xpg": Pg[c * gx:(c + 1) * gx],
            "Xpm": Pm[c * mx:(c + 1) * mx],
        })
    res = bass_utils.run_bass_kernel_spmd(
        nc, in_maps, core_ids=list(range(NC)), trace=False)
    rs = res.results

    scores = np.concatenate([rs[c]["scores"] for c in range(NC)], axis=0)

    def gather_v(name):
        v = np.concatenate([rs[c][name] for c in range(NC)], axis=0)
        return v.astype(np.int32).reshape(-1)

    u_g = np.repeat(np.arange(n_graphs, dtype=np.int32), KNN_K)
    u_m = np.repeat(np.arange(n_models, dtype=np.int32), KNN_K)

    out = (scores,
           u_g, gather_v("v_mg2g"),
           u_g, gather_v("v_pg2g"),
           u_g, gather_v("v_pg2m"),
           u_m, gather_v("v_pm2g"),
           u_m, gather_v("v_pm2m"))
    if _trace:
        return out, res
    return out


def time_kernel(M, Pg, Pm, W, b, iters=8):
    """Time device execution with inputs resident on-device (pipelined and
    per-call-blocked); returns (pipelined_ns, blocked_ns) per execution."""
    import jax
    from jax.sharding import Mesh, PartitionSpec
    from jax.experimental.shard_map import shard_map
    import concourse.bass2jax as b2j
    import concourse.mybir as mb

    n_graphs, n_models = M.shape[0], Pm.shape[0]
    gx, mx = n_graphs // NC, n_models // NC
    nc = _get_program(n_graphs, n_models)
    b2j.install_neuronx_cc_hook()

    in_names, out_names, out_avals, zero_outs = [], [], [], []
    pname = nc.partition_id_tensor.name if nc.partition_id_tensor else None
    for alloc in nc.m.functions[0].allocations:
        if not isinstance(alloc, mb.MemoryLocationSet):
            continue
        name = alloc.memorylocations[0].name
        if alloc.kind == "ExternalInput":
            if name != pname:
                in_names.append(name)
        elif alloc.kind == "ExternalOutput":
            out_names.append(name)
            shape = tuple(alloc.tensor_shape)
            dtype = mb.dt.np(alloc.dtype)
            out_avals.append(jax.core.ShapedArray(shape, dtype))
            zero_outs.append(np.zeros(shape, dtype))
    n_params = len(in_names)
    all_in_names = in_names + out_names
    if pname is not None:
        all_in_names = all_in_names + [pname]

    def _body(*args):
        operands = list(args)
        if pname is not None:
            operands.append(b2j.partition_id_tensor())
        return tuple(b2j._bass_exec_p.bind(
            *operands, out_avals=tuple(out_avals), in_names=tuple(all_in_names),
            out_names=tuple(out_names), lowering_input_output_aliases=(),
            sim_require_finite=True, sim_require_nnan=True, nc=nc))

    devices = jax.devices()[:NC]
    mesh = Mesh(np.asarray(devices), ("core",))
    in_specs = (PartitionSpec("core"),) * (n_params + len(out_names))
    out_specs = (PartitionSpec("core"),) * len(out_names)
    fn = jax.jit(shard_map(_body, mesh=mesh, in_specs=in_specs,
                           out_specs=out_specs, check_rep=False),
                 keep_unused=True)

    per_core_in = {
        "M": lambda c: M, "Pg": lambda c: Pg, "Pm": lambda c: Pm,
        "W": lambda c: W, "b": lambda c: b,
        "Xm": lambda c: M[c * gx:(c + 1) * gx],
        "Xpg": lambda c: Pg[c * gx:(c + 1) * gx],
        "Xpm": lambda c: Pm[c * mx:(c + 1) * mx],
    }
    concat_in = [np.concatenate([np.asarray(per_core_in[n](c))
                                 for c in range(NC)], axis=0)
                 for n in in_names]
    concat_zero = [np.zeros((NC * z.shape[0], *z.shape[1:]), z.dtype)
                   for z in zero_outs]
    sharding = jax.sharding.NamedSharding(mesh, PartitionSpec("core"))
    dev_in = [jax.device_put(x, sharding) for x in concat_in + concat_zero]

    outs = fn(*dev_in)   # compile + warmup
    jax.block_until_ready(outs)

    t0 = time.perf_counter()
    all_outs = [fn(*dev_in) for _ in range(iters)]
    jax.block_until_ready(all_outs)
    pipelined_ns = (time.perf_counter() - t0) / iters * 1e9

    blocked = []
    for _ in range(iters):
        t0 = time.perf_counter()
        jax.block_until_ready(fn(*dev_in))
        blocked.append((time.perf_counter() - t0) * 1e9)
    return pipelined_ns, min(blocked)
